# revision 1
# baseline (speedup 1.0000x reference)
"""Trainium2 Bass kernel for MFA (mixture of factor analyzers) log-prob.

Data-parallel over N across 8 NeuronCores. Host folds the Woodbury/Cholesky
algebra into three small weight matrices so each device computes, per sample:
    comp^T = w3^T @ xsq^T + w2^T @ x^T + ind^T @ (Wc^T @ x^T)^2 + off
(4 matmuls over the d=128 feature/partition dim, one elementwise square),
then a sample-major logsumexp over K=32 components.

x is pre-transposed on the host so DMA delivers feature-major tiles
[128 features, n samples] with contiguous per-partition descriptors; the
[128, cols] output is un-permuted on the host.
"""

import math
from contextlib import ExitStack

import numpy as np

import concourse.bass as bass
import concourse.bacc as bacc
import concourse.mybir as mybir
import concourse.tile as tile
from concourse.bass_utils import run_bass_kernel_spmd

N_TOTAL = 500000
D = 128
K = 32
L = 4
N_CORES = 8
N_PER_CORE = N_TOTAL // N_CORES           # 62500
MACRO = 512                               # samples per macro-tile
SUPER = 4                                 # macro-tiles per logsumexp batch
_nm = (N_PER_CORE + MACRO - 1) // MACRO
N_MACROS = ((_nm + SUPER - 1) // SUPER) * SUPER   # 124
N_PAD = N_MACROS * MACRO                  # 63488
N_COLS = N_PAD // 128                     # 496

FP32 = mybir.dt.float32
FP32R = mybir.dt.float32r


def _factorize(MU, A, D_, PI):
    Kk, d, l = A.shape
    MU = MU.astype(np.float64)
    A = A.astype(np.float64)
    D_ = D_.astype(np.float64)
    PI = PI.astype(np.float64)

    iD = D_ ** -2.0
    B = iD[:, :, None] * A
    Lm = np.eye(l)[None] + np.einsum('kdl,kdm->klm', A, B)
    iL = np.linalg.inv(Lm)
    C = np.linalg.cholesky(iL)
    W0 = np.einsum('kdl,klm->kdm', B, C)              # [K,d,l]
    c = np.einsum('kd,kdl->kl', MU, W0)

    w3 = -0.5 * iD.T                                  # [d,K]
    w2 = (iD * MU).T - np.einsum('kl,kdl->dk', c, W0)
    Wc = (W0 * math.sqrt(0.5)).transpose(1, 0, 2).reshape(d, Kk * l)
    logdet = np.log(np.linalg.det(Lm)) + np.sum(np.log(D_ ** 2), axis=1)
    t_const = np.sum(iD * MU * MU, axis=1)
    off = PI - 0.5 * (d * math.log(2 * math.pi) + logdet + t_const) \
        + 0.5 * np.sum(c * c, axis=1)
    return (np.ascontiguousarray(Wc, dtype=np.float32),
            np.ascontiguousarray(w2, dtype=np.float32),
            np.ascontiguousarray(w3, dtype=np.float32),
            off.astype(np.float32))


def _build_bass():
    nc = bacc.Bacc(None, target_bir_lowering=False)

    xT = nc.dram_tensor("xT", [D, N_PAD], FP32R, kind="ExternalInput")
    big_d = nc.dram_tensor("bigc", [D, K * L + 3 * K], FP32R, kind="ExternalInput")
    off_d = nc.dram_tensor("off", [K, 1], FP32, kind="ExternalInput")
    id_d = nc.dram_tensor("ident", [K, K], FP32R, kind="ExternalInput")
    y_d = nc.dram_tensor("y", [128, N_COLS], FP32, kind="ExternalOutput")

    with tile.TileContext(nc) as tc, ExitStack() as ctx:
        consts = ctx.enter_context(tc.tile_pool(name="consts", bufs=1))
        xpool = ctx.enter_context(tc.tile_pool(name="xpool", bufs=4))
        spool = ctx.enter_context(tc.tile_pool(name="spool", bufs=3))
        lsepool = ctx.enter_context(tc.tile_pool(name="lsepool", bufs=2))
        respool = ctx.enter_context(tc.tile_pool(name="respool", bufs=1))
        psUp = ctx.enter_context(tc.tile_pool(name="psU", bufs=2, space="PSUM"))
        psCp = ctx.enter_context(tc.tile_pool(name="psC", bufs=2, space="PSUM"))
        psDp = ctx.enter_context(tc.tile_pool(name="psD", bufs=2, space="PSUM"))

        sb_big = consts.tile([D, K * L + 3 * K], FP32R)
        sb_off = consts.tile([K, 1], FP32)
        sb_id = consts.tile([K, K], FP32R)
        nc.sync.dma_start(out=sb_big, in_=big_d[:, :])
        nc.sync.dma_start(out=sb_off, in_=off_d[:, :])
        nc.sync.dma_start(out=sb_id, in_=id_d[:, :])
        sb_wc = sb_big[:, 0:K * L]
        sb_w2 = sb_big[:, K * L:K * L + K]
        sb_w3 = sb_big[:, K * L + K:K * L + 2 * K]
        sb_ind = sb_big[:, K * L + 2 * K:K * L + 3 * K]

        resbuf = respool.tile([128, N_COLS], FP32)

        for s in range(N_MACROS // SUPER):
            ps_comp = psDp.tile([128, SUPER * 4, K], FP32R, tag="psD")
            for i in range(SUPER):
                t = s * SUPER + i
                sb_x = xpool.tile([D, MACRO], FP32R, tag="x")
                nc.sync.dma_start(out=sb_x, in_=xT[:, t * MACRO:(t + 1) * MACRO])

                sb_xsq = spool.tile([D, MACRO], FP32R, tag="xsq")
                nc.vector.tensor_mul(sb_xsq, sb_x, sb_x)

                ps_u = psUp.tile([K * L, MACRO], FP32, tag="u")
                nc.tensor.matmul(
                    ps_u, sb_wc, sb_x,
                    start=True, stop=True)

                sb_usq = spool.tile([K * L, MACRO], FP32R, tag="usq")
                nc.scalar.activation(
                    out=sb_usq, in_=ps_u,
                    func=mybir.ActivationFunctionType.Square)

                ps_c = psCp.tile([K, MACRO], FP32, tag="c")
                nc.tensor.matmul(
                    ps_c, sb_w3, sb_xsq,
                    start=True, stop=False)
                nc.tensor.matmul(
                    ps_c, sb_w2, sb_x,
                    start=False, stop=False)
                nc.tensor.matmul(
                    ps_c, sb_ind, sb_usq,
                    start=False, stop=True)

                sb_comp = spool.tile([K, MACRO], FP32R, tag="comp")
                nc.scalar.activation(
                    out=sb_comp, in_=ps_c,
                    func=mybir.ActivationFunctionType.Identity,
                    bias=sb_off, scale=1.0)

                for j in range(MACRO // 128):
                    nc.tensor.transpose(
                        ps_comp[:, i * 4 + j, :],
                        sb_comp[:, j * 128:(j + 1) * 128],
                        sb_id)

            n_grp = SUPER * 4
            mx = lsepool.tile([128, n_grp], FP32, tag="mx")
            nc.vector.reduce_max(mx, ps_comp, axis=mybir.AxisListType.X)
            sb_e = lsepool.tile([128, n_grp, K], FP32, tag="e")
            nc.vector.tensor_sub(
                sb_e, ps_comp,
                mx.unsqueeze(2).broadcast_to([128, n_grp, K]))
            nc.scalar.activation(
                out=sb_e, in_=sb_e, func=mybir.ActivationFunctionType.Exp)
            ssum = lsepool.tile([128, n_grp], FP32, tag="ssum")
            nc.vector.reduce_sum(ssum, sb_e, axis=mybir.AxisListType.X)
            lse = lsepool.tile([128, n_grp], FP32, tag="lse")
            nc.scalar.activation(
                out=lse, in_=ssum, func=mybir.ActivationFunctionType.Ln)
            nc.vector.tensor_add(
                resbuf[:, s * n_grp:(s + 1) * n_grp], lse, mx)

        nc.sync.dma_start(out=y_d[:, :], in_=resbuf)

    nc.compile()
    return nc


_CACHE = {}


def kernel(x, MU, A, D, PI):
    Wc, w2, w3, off = _factorize(MU, A, D, PI)
    ind = np.zeros((K * L, K), dtype=np.float32)
    for k in range(K):
        ind[k * L:(k + 1) * L, k] = 1.0
    ident = np.eye(K, dtype=np.float32)

    bigc = np.concatenate([Wc, w2, w3, ind], axis=1)

    if "nc" not in _CACHE:
        _CACHE["nc"] = _build_bass()
    nc = _CACHE["nc"]

    in_maps = []
    for c in range(N_CORES):
        shard = np.asarray(x[c * N_PER_CORE:(c + 1) * N_PER_CORE],
                           dtype=np.float32)
        xTs = np.zeros((128, N_PAD), dtype=np.float32)
        xTs[:, :N_PER_CORE] = shard.T
        in_maps.append({
            "xT": xTs, "bigc": bigc,
            "off": off.reshape(K, 1), "ident": ident,
        })

    import os
    trace = bool(int(os.environ.get("MFA_TRACE", "0")))
    if trace:
        try:
            res = run_bass_kernel_spmd(nc, in_maps,
                                       core_ids=list(range(N_CORES)),
                                       trace=True)
            print(f"HW exec time: {res.exec_time_ns} ns", flush=True)
            if res.instructions_and_trace is not None:
                print(f"trace: {res.instructions_and_trace[1]}", flush=True)
        except Exception as e:
            print(f"trace unavailable ({e}); rerunning untraced", flush=True)
            res = run_bass_kernel_spmd(nc, in_maps,
                                       core_ids=list(range(N_CORES)))
    else:
        res = run_bass_kernel_spmd(nc, in_maps, core_ids=list(range(N_CORES)))
    outs = []
    for c in range(N_CORES):
        y_dev = res.results[c]["y"]
        outs.append(y_dev.T.reshape(-1)[:N_PER_CORE])
    return np.concatenate(outs).astype(np.float32)



# revision 4
# speedup vs baseline: 2.0947x; 2.0947x over previous
"""Trainium2 Bass kernel for MFA (mixture of factor analyzers) log-prob.

Data-parallel over N across 8 NeuronCores. Host folds the Woodbury/Cholesky
algebra into three small weight matrices so each device computes, per sample:
    comp^T = w3^T @ xsq^T + w2^T @ x^T + ind^T @ (Wc^T @ x^T)^2 + off
(4 matmuls over the d=128 feature/partition dim, one elementwise square),
then a sample-major logsumexp over K=32 components.

x is shipped sample-major as fp16 (halves the host->device transfer, which
dominates wall time) and transposed to feature-major on-device with the PE
array. Cores run identical programs over overlapping 123*512-sample windows
so no host-side padding or scatter pass is needed.
"""

import math
from contextlib import ExitStack

import numpy as np

import concourse.bass as bass
import concourse.bacc as bacc
import concourse.mybir as mybir
import concourse.tile as tile
from concourse.bass_utils import run_bass_kernel_spmd

N_TOTAL = 500000
D = 128
K = 32
L = 4
N_CORES = 8
N_PER_CORE = N_TOTAL // N_CORES           # 62500
MACRO = 512                               # samples per macro-tile
SUPER = 3                                 # macro-tiles per logsumexp batch
N_MACROS = 123                            # 123*512 = 62976 >= 62500
N_SPAN = N_MACROS * MACRO                 # samples each core computes
N_COLS = N_SPAN // 128                    # 492
N_WIN = 352                               # fp16 weight columns: Wc|w3|w2|ind|id

FP32 = mybir.dt.float32
FP16 = mybir.dt.float16


def _factorize(MU, A, D_, PI):
    Kk, d, l = A.shape
    MU = MU.astype(np.float64)
    A = A.astype(np.float64)
    D_ = D_.astype(np.float64)
    PI = PI.astype(np.float64)

    iD = D_ ** -2.0
    B = iD[:, :, None] * A
    Lm = np.eye(l)[None] + np.einsum('kdl,kdm->klm', A, B)
    iL = np.linalg.inv(Lm)
    C = np.linalg.cholesky(iL)
    W0 = np.einsum('kdl,klm->kdm', B, C)              # [K,d,l]
    c = np.einsum('kd,kdl->kl', MU, W0)

    w3 = -0.5 * iD.T                                  # [d,K]
    w2 = (iD * MU).T - np.einsum('kl,kdl->dk', c, W0)
    Wc = (W0 * math.sqrt(0.5)).transpose(1, 0, 2).reshape(d, Kk * l)
    logdet = np.log(np.linalg.det(Lm)) + np.sum(np.log(D_ ** 2), axis=1)
    t_const = np.sum(iD * MU * MU, axis=1)
    off = PI - 0.5 * (d * math.log(2 * math.pi) + logdet + t_const) \
        + 0.5 * np.sum(c * c, axis=1)
    return (np.ascontiguousarray(Wc, dtype=np.float32),
            np.ascontiguousarray(w2, dtype=np.float32),
            np.ascontiguousarray(w3, dtype=np.float32),
            off.astype(np.float32))


def _build_bass():
    nc = bacc.Bacc(None, target_bir_lowering=False)

    xN = nc.dram_tensor("xN", [N_SPAN, D], FP16, kind="ExternalInput")
    w_d = nc.dram_tensor("wts", [D, N_WIN], FP16, kind="ExternalInput")
    off_d = nc.dram_tensor("off", [K, 1], FP32, kind="ExternalInput")
    y_d = nc.dram_tensor("y", [128, N_COLS], FP32, kind="ExternalOutput")

    with tile.TileContext(nc) as tc, ExitStack() as ctx:
        consts = ctx.enter_context(tc.tile_pool(name="consts", bufs=1))
        xpool = ctx.enter_context(tc.tile_pool(name="xpool", bufs=3))
        xtpool = ctx.enter_context(tc.tile_pool(name="xtpool", bufs=2))
        spool = ctx.enter_context(tc.tile_pool(name="spool", bufs=3))
        lsepool = ctx.enter_context(tc.tile_pool(name="lsepool", bufs=2))
        respool = ctx.enter_context(tc.tile_pool(name="respool", bufs=1))
        psTp = ctx.enter_context(tc.tile_pool(name="psT", bufs=2, space="PSUM"))
        psUp = ctx.enter_context(tc.tile_pool(name="psU", bufs=2, space="PSUM"))
        psCp = ctx.enter_context(tc.tile_pool(name="psC", bufs=2, space="PSUM"))
        psDp = ctx.enter_context(tc.tile_pool(name="psD", bufs=2, space="PSUM"))

        sb_w = consts.tile([D, N_WIN], FP16)
        sb_off = consts.tile([K, 1], FP32)
        nc.sync.dma_start(out=sb_w, in_=w_d[:, :])
        nc.sync.dma_start(out=sb_off, in_=off_d[:, :])
        sb_wc = sb_w[:, 0:K * L]
        sb_w3 = sb_w[:, K * L:K * L + K]
        sb_w2 = sb_w[:, K * L + K:K * L + 2 * K]
        sb_ind = sb_w[:, K * L + 2 * K:K * L + 3 * K]
        sb_id = sb_w[:, K * L + 3 * K:K * L + 3 * K + 128]

        resbuf = respool.tile([128, N_COLS], FP32)

        # psum->sbuf copy engines alternate to balance load
        cp_engines = None

        for s in range(N_MACROS // SUPER):
            ps_ct = psDp.tile([128, SUPER * 4, K], FP16, tag="psD")
            for i in range(SUPER):
                t = s * SUPER + i
                sb_x = xpool.tile([128, 4, D], FP16, tag="x")
                for j in range(4):
                    nc.sync.dma_start(
                        out=sb_x[:, j, :],
                        in_=xN[t * MACRO + j * 128:t * MACRO + (j + 1) * 128, :])

                sb_xT = xtpool.tile([D, MACRO], FP16, tag="xT")
                for j in range(4):
                    ps_t = psTp.tile([128, 128], FP16, tag="pst")
                    nc.tensor.transpose(ps_t, sb_x[:, j, :], sb_id)
                    if j == 3:
                        nc.scalar.copy(
                            out=sb_xT[:, j * 128:(j + 1) * 128], in_=ps_t)
                    else:
                        nc.vector.tensor_copy(
                            sb_xT[:, j * 128:(j + 1) * 128], ps_t)

                sb_xsq = spool.tile([D, MACRO], FP16, tag="xsq")
                nc.gpsimd.tensor_mul(sb_xsq, sb_xT, sb_xT)

                ps_u = psUp.tile([K * L, MACRO], FP32, tag="u")
                nc.tensor.matmul(ps_u, sb_wc, sb_xT, start=True, stop=True)

                sb_usq = spool.tile([K * L, MACRO], FP16, tag="usq")
                nc.scalar.activation(
                    out=sb_usq, in_=ps_u,
                    func=mybir.ActivationFunctionType.Square)

                ps_c = psCp.tile([K, MACRO], FP32, tag="c")
                nc.tensor.matmul(ps_c, sb_w3, sb_xsq, start=True, stop=False)
                nc.tensor.matmul(ps_c, sb_w2, sb_xT, start=False, stop=False)
                nc.tensor.matmul(ps_c, sb_ind, sb_usq, start=False, stop=True)

                sb_comp = spool.tile([K, MACRO], FP16, tag="comp")
                nc.scalar.activation(
                    out=sb_comp, in_=ps_c,
                    func=mybir.ActivationFunctionType.Identity,
                    bias=sb_off, scale=1.0)

                for j in range(4):
                    nc.tensor.transpose(
                        ps_ct[:, i * 4 + j, :],
                        sb_comp[:, j * 128:(j + 1) * 128],
                        sb_id[0:K, 0:K])

            n_grp = SUPER * 4
            mx = lsepool.tile([128, n_grp], FP32, tag="mx")
            nc.vector.reduce_max(mx, ps_ct, axis=mybir.AxisListType.X)
            sb_e = lsepool.tile([128, n_grp, K], FP32, tag="e")
            nc.vector.tensor_sub(
                sb_e, ps_ct,
                mx.unsqueeze(2).broadcast_to([128, n_grp, K]))
            nc.scalar.activation(
                out=sb_e, in_=sb_e, func=mybir.ActivationFunctionType.Exp)
            ssum = lsepool.tile([128, n_grp], FP32, tag="ssum")
            nc.vector.reduce_sum(ssum, sb_e, axis=mybir.AxisListType.X)
            lse = lsepool.tile([128, n_grp], FP32, tag="lse")
            nc.scalar.activation(
                out=lse, in_=ssum, func=mybir.ActivationFunctionType.Ln)
            nc.vector.tensor_add(
                resbuf[:, s * n_grp:(s + 1) * n_grp], lse, mx)

        nc.sync.dma_start(out=y_d[:, :], in_=resbuf)

    nc.compile()
    return nc


_CACHE = {}


def kernel(x, MU, A, D, PI):
    Wc, w2, w3, off = _factorize(MU, A, D, PI)
    ind = np.zeros((K * L, K), dtype=np.float32)
    for k in range(K):
        ind[k * L:(k + 1) * L, k] = 1.0
    ident = np.eye(128, dtype=np.float32)
    wts = np.concatenate([Wc, w3, w2, ind, ident],
                         axis=1).astype(np.float16)

    if "nc" not in _CACHE:
        _CACHE["nc"] = _build_bass()
    nc = _CACHE["nc"]

    x16 = np.asarray(x, dtype=np.float16)
    starts = [min(c * N_PER_CORE, N_TOTAL - N_SPAN) for c in range(N_CORES)]
    in_maps = []
    for c in range(N_CORES):
        in_maps.append({
            "xN": x16[starts[c]:starts[c] + N_SPAN],
            "wts": wts,
            "off": off.reshape(K, 1),
        })

    res = run_bass_kernel_spmd(nc, in_maps, core_ids=list(range(N_CORES)))

    out = np.empty(N_TOTAL, dtype=np.float32)
    for c in range(N_CORES):
        yc = res.results[c]["y"].T.reshape(-1)
        o = c * N_PER_CORE - starts[c]
        out[c * N_PER_CORE:(c + 1) * N_PER_CORE] = yc[o:o + N_PER_CORE]
    return out
